# revision 3
# baseline (speedup 1.0000x reference)
"""Trainium2 Bass kernel for nn_DigitLayer (CapsNet digit-capsule layer).

Math: the reference's routing softmax acts on a size-1 axis, so coupling
coefficients are exactly 1.0 and the 3-iteration routing loop collapses to

    S[b,d,i] = sum_{p,j} W[p,d,i,j] * x[b,p,j];  out = squash(S) over i

i.e. one [B, P*8] @ [P*8, 160] matmul + a per-(b,d) squash. The contraction
dim P is sharded across the 8 cores (every byte of x and W read from HBM
exactly once chip-wide, ~0.96MB f16 per core); the host sums the 8 partial
S tensors and applies the squash.

Performance structure (from NTFF traces; ~5.8us fixed NEFF startup and
~0.8us fixed end-of-program barrier bound everything):

  * ONE combined input tensor per core, in_t [128, 9, 416] f16: each
    k-chunk line carries that chunk's x block (256 cols) and w block
    (160 cols) contiguously, so one DMA gates both operands of a chunk.
  * DMA plan: almost everything on the ACT HWDGE ring (a single queue
    sustains ~300+ GB/s with multi-KB lines; two competing queues drop to
    ~220 GB/s aggregate), chunk-group gates sized so the PE never waits
    long, one mid chunk on the gpsimd SWDGE queue for extra issue
    bandwidth (its ~3.7us end-to-end latency only tolerates mid-stream
    placement), and a tiny final gate so the PE tail after the last DMA
    sem (+900ns propagation) is ~2 matmuls.
  * The framework's init all-engine barrier is skipped (LeanBacc): its
    per-engine InstDrain waits for DMA-queue drain, which would serialize
    the program behind the SWDGE input DMA; nothing here needs it (const
    memsets have no consumers, user ops are semaphore-gated).
  * PE pre-warm: ~26 dummy matmuls into a scratch PSUM bank while input
    streams in. The PE DVFS ramps LOW->MID->FULL after ~4.2us of sustained
    activity; warmed, the real 18 matmuls run at 69ns instead of 133ns.
  * Output: DVE casts PSUM bank 0 and the ACT engine casts bank 1 (in
    parallel) into one osb [128, 320] f16 buffer; a single 640B-line DMA
    stores it. The host undoes the [128, 2*160] layout, sums partials in
    fp32, and squashes. f16 end-to-end keeps rel err ~5e-4 (gate is 2e-2).
"""

import numpy as np

import concourse.bacc as bacc
import concourse.mybir as mybir
from concourse.bass_utils import run_bass_kernel_spmd

B, P, D, VP, VD = 256, 1152, 10, 8, 16
NCORES = 8
PL = P // NCORES
KL = PL * VP               # 1152
KCH = KL // 128            # 9
N_OUT = D * VD             # 160
MB = 128
NMB = B // MB              # 2
CW = MB * NMB + N_OUT      # 416

# (engine, [chunks]); per-engine emission order = list order. PE consumes
# chunks c0..c8 in gate-list order.
PLAN = [
    ("scalar", [0, 1, 2]),
    ("pool", [3]),
    ("scalar", [4, 5, 6]),
    ("scalar", [7, 8]),
]

class LeanBacc(bacc.Bacc):
    """Bacc whose framework init all-engine barrier can be skipped once.

    The init barrier's per-engine InstDrain waits for the engine's DMA queue
    to drain -- with a hoisted SWDGE input DMA in flight that serializes the
    whole program behind its transfer. Nothing downstream needs the barrier:
    the const memsets have no consumers here and user instructions are gated
    by their own semaphores."""

    _skip_barrier_once = False

    def all_engine_barrier(self, *, sem_only=False):
        if LeanBacc._skip_barrier_once:
            LeanBacc._skip_barrier_once = False
            return
        return super().all_engine_barrier(sem_only=sem_only)


_cache = {}


def _hoist_first(nc, instrs):
    names = {i.name for i in instrs}
    for bb in nc.main_func.blocks:
        if not any(ins.name in names for ins in bb.instructions):
            continue
        by_engine = {}
        for ins in bb.instructions:
            if ins.name in names:
                by_engine.setdefault(ins.engine, []).append(ins)
        new = []
        emitted = set()
        for ins in bb.instructions:
            if ins.name in names:
                continue
            e = ins.engine
            if e in by_engine and e not in emitted:
                new.extend(by_engine[e])
                emitted.add(e)
            new.append(ins)
        for e, lst in by_engine.items():
            if e not in emitted:
                new.extend(lst)
        bb.instructions[:] = new


def _build():
    dt_in = mybir.dt.float16
    LeanBacc._skip_barrier_once = True
    nc = LeanBacc("TRN2", debug=False, num_devices=NCORES)
    in_t = nc.dram_tensor("in_t", [128, KCH, CW], dt_in, kind="ExternalInput").ap()
    out_t = nc.dram_tensor("out", [128, NMB * N_OUT], dt_in, kind="ExternalOutput").ap()

    from contextlib import ExitStack
    with ExitStack() as ctx:
        insb = ctx.enter_context(nc.sbuf_tensor([128, KCH, CW], dt_in))
        osb = ctx.enter_context(nc.sbuf_tensor([128, NMB * N_OUT], dt_in))
        pts = [
            ctx.enter_context(nc.psum_tensor(f"pt{m}", [MB, N_OUT], mybir.dt.float32))
            for m in range(NMB)
        ]
        ptw = ctx.enter_context(nc.psum_tensor("ptw", [MB, N_OUT], mybir.dt.float32))
        sem_gs = [ctx.enter_context(nc.semaphore(name=f"sem_g{g}"))
                  for g in range(len(PLAN))]
        sem_mm = ctx.enter_context(nc.semaphore(name="sem_mm"))
        sem_cp = ctx.enter_context(nc.semaphore(name="sem_cp"))
        sem_out = ctx.enter_context(nc.semaphore(name="sem_out"))

        engines = {"scalar": nc.scalar, "sync": nc.sync, "pool": nc.gpsimd}
        in_dmas = []
        for g, (eng, chunks) in enumerate(PLAN):
            k0, k1 = chunks[0], chunks[-1] + 1
            assert chunks == list(range(k0, k1))
            in_dmas.append(engines[eng].dma_start(
                out=insb[:, k0:k1, :], in_=in_t[:, k0:k1, :]
            ).then_inc(sem_gs[g], 16).ins)

        # PE pre-warm: dummy matmuls on (uninitialized) SBUF into a scratch
        # PSUM bank while waiting for the first gate, to ramp the PE p-state
        # before the real accumulation starts.
        for _ in range(26):
            nc.tensor.matmul(
                ptw[:], lhsT=insb[:, 0, 0:MB], rhs=insb[:, 0, NMB * MB:],
                start=True, stop=True, skip_group_check=True,
            )

        first_chunk = PLAN[0][1][0]
        last_chunk = PLAN[-1][1][-1]
        for g, (eng, chunks) in enumerate(PLAN):
            nc.tensor.wait_ge(sem_gs[g], 16)
            for k in chunks:
                for m in range(NMB):
                    mm = nc.tensor.matmul(
                        pts[m][:],
                        lhsT=insb[:, k, m * MB:(m + 1) * MB],
                        rhs=insb[:, k, NMB * MB:],
                        start=(k == first_chunk),
                        stop=(k == last_chunk),
                    )
                    if k == last_chunk:
                        if m == 0:
                            mm.then_inc(sem_mm, 1)
                        else:
                            nc.tensor.drain().then_inc(sem_mm, 1)

        # half-0 cast on DVE (overlaps the PE's half-1 tail), half-1 on the
        # ACT engine so the two casts run in parallel.
        nc.vector.wait_ge(sem_mm, 1)
        nc.vector.tensor_copy(osb[:, 0:N_OUT], pts[0][:])
        nc.vector.drain().then_inc(sem_cp, 1)
        nc.scalar.wait_ge(sem_mm, 2)
        nc.scalar.copy(osb[:, N_OUT:2 * N_OUT], pts[1][:])
        nc.scalar.drain().then_inc(sem_cp, 1)

        nc.sync.wait_ge(sem_cp, 2)
        nc.sync.dma_start(out=out_t[:], in_=osb[:]).then_inc(sem_out, 16)

        _hoist_first(nc, in_dmas)
    nc.compile()
    return nc


def _prep_inputs(x, W):
    xs = np.ascontiguousarray(x[..., 0], dtype=np.float32)
    W = np.asarray(W, dtype=np.float32)
    in_maps = []
    for c in range(NCORES):
        pr = slice(c * PL, (c + 1) * PL)
        xl = xs[:, pr, :].reshape(B, KL).T
        xl = xl.reshape(KCH, 128, B).transpose(1, 0, 2)
        wl = W[pr].transpose(0, 3, 1, 2).reshape(KL, N_OUT)
        wl = wl.reshape(KCH, 128, N_OUT).transpose(1, 0, 2)
        comb = np.concatenate([xl, wl], axis=2)
        in_maps.append({"in_t": np.ascontiguousarray(comb, dtype=np.float16)})
    return in_maps


def _squash(S):
    S = S.reshape(B, D, VD)
    sq = np.sum(S * S, axis=2, keepdims=True)
    v = S * sq / (1.0 + sq) / np.sqrt(sq + 1e-9)
    return v[..., None].astype(np.float32)


def run(x, W, trace=False):
    if "nc" not in _cache:
        _cache["nc"] = _build()
    nc = _cache["nc"]
    in_maps = _prep_inputs(x, W)
    try:
        res = run_bass_kernel_spmd(nc, in_maps, core_ids=list(range(NCORES)), trace=trace)
    except Exception:
        res = run_bass_kernel_spmd(nc, in_maps, core_ids=list(range(NCORES)), trace=trace)
    S = np.zeros((B, N_OUT), dtype=np.float32)
    for c in range(NCORES):
        o = res.results[c]["out"].astype(np.float32)
        S[:MB] += o[:, :N_OUT]
        S[MB:] += o[:, N_OUT:]
    return _squash(S), res


def kernel(x, W):
    out, _ = run(np.asarray(x), np.asarray(W))
    return out


# revision 4
# speedup vs baseline: 1.0237x; 1.0237x over previous
"""Trainium2 Bass kernel for nn_DigitLayer (CapsNet digit-capsule layer).

Math: the reference's routing softmax acts on a size-1 axis, so coupling
coefficients are exactly 1.0 and the 3-iteration routing loop collapses to

    S[b,d,i] = sum_{p,j} W[p,d,i,j] * x[b,p,j];  out = squash(S) over i

i.e. one [B, P*8] @ [P*8, 160] matmul + a per-(b,d) squash. The contraction
dim P is sharded across the 8 cores (every byte of x and W read from HBM
exactly once chip-wide, ~0.96MB f16 per core); the host sums the 8 partial
S tensors and applies the squash.

Performance structure (from NTFF traces; ~5.8us fixed NEFF startup and
~0.8us fixed end-of-program barrier bound everything):

  * ONE combined input tensor per core, in_t [128, 9, 416] f16: each
    k-chunk line carries that chunk's x block (256 cols) and w block
    (160 cols) contiguously, so one DMA gates both operands of a chunk.
  * DMA plan: almost everything on the ACT HWDGE ring (a single queue
    sustains ~300+ GB/s with multi-KB lines; two competing queues drop to
    ~220 GB/s aggregate), chunk-group gates sized so the PE never waits
    long, one mid chunk on the gpsimd SWDGE queue for extra issue
    bandwidth (its ~3.7us end-to-end latency only tolerates mid-stream
    placement), and two tiny single-chunk final gates so the PE
    tail after the last DMA sem (+900ns propagation) is ~2 matmuls.
  * The framework's init all-engine barrier is skipped (LeanBacc): its
    per-engine InstDrain waits for DMA-queue drain, which would serialize
    the program behind the SWDGE input DMA; nothing here needs it (const
    memsets have no consumers, user ops are semaphore-gated).
  * PE pre-warm: ~26 dummy matmuls into a scratch PSUM bank while input
    streams in. The PE DVFS ramps LOW->MID->FULL after ~4.2us of sustained
    activity; warmed, the real 18 matmuls run at 69ns instead of 133ns.
  * Output: DVE casts PSUM bank 0 and the ACT engine casts bank 1 (in
    parallel) into one osb [128, 320] f16 buffer; a single 640B-line DMA
    stores it. The host undoes the [128, 2*160] layout, sums partials in
    fp32, and squashes. f16 end-to-end keeps rel err ~5e-4 (gate is 2e-2).
"""

import numpy as np

import concourse.bacc as bacc
import concourse.mybir as mybir
from concourse.bass_utils import run_bass_kernel_spmd

B, P, D, VP, VD = 256, 1152, 10, 8, 16
NCORES = 8
PL = P // NCORES
KL = PL * VP               # 1152
KCH = KL // 128            # 9
N_OUT = D * VD             # 160
MB = 128
NMB = B // MB              # 2
CW = MB * NMB + N_OUT      # 416

# (engine, [chunks]); per-engine emission order = list order. PE consumes
# chunks c0..c8 in gate-list order.
PLAN = [
    ("scalar", [0, 1, 2]),
    ("pool", [3]),
    ("scalar", [4, 5, 6]),
    ("scalar", [7]),
    ("scalar", [8]),
]

class LeanBacc(bacc.Bacc):
    """Bacc whose framework init all-engine barrier can be skipped once.

    The init barrier's per-engine InstDrain waits for the engine's DMA queue
    to drain -- with a hoisted SWDGE input DMA in flight that serializes the
    whole program behind its transfer. Nothing downstream needs the barrier:
    the const memsets have no consumers here and user instructions are gated
    by their own semaphores."""

    _skip_barrier_once = False

    def all_engine_barrier(self, *, sem_only=False):
        if LeanBacc._skip_barrier_once:
            LeanBacc._skip_barrier_once = False
            return
        return super().all_engine_barrier(sem_only=sem_only)


_cache = {}


def _hoist_first(nc, instrs):
    names = {i.name for i in instrs}
    for bb in nc.main_func.blocks:
        if not any(ins.name in names for ins in bb.instructions):
            continue
        by_engine = {}
        for ins in bb.instructions:
            if ins.name in names:
                by_engine.setdefault(ins.engine, []).append(ins)
        new = []
        emitted = set()
        for ins in bb.instructions:
            if ins.name in names:
                continue
            e = ins.engine
            if e in by_engine and e not in emitted:
                new.extend(by_engine[e])
                emitted.add(e)
            new.append(ins)
        for e, lst in by_engine.items():
            if e not in emitted:
                new.extend(lst)
        bb.instructions[:] = new


def _build():
    dt_in = mybir.dt.float16
    LeanBacc._skip_barrier_once = True
    nc = LeanBacc("TRN2", debug=False, num_devices=NCORES)
    in_t = nc.dram_tensor("in_t", [128, KCH, CW], dt_in, kind="ExternalInput").ap()
    out_t = nc.dram_tensor("out", [128, NMB * N_OUT], dt_in, kind="ExternalOutput").ap()

    from contextlib import ExitStack
    with ExitStack() as ctx:
        insb = ctx.enter_context(nc.sbuf_tensor([128, KCH, CW], dt_in))
        osb = ctx.enter_context(nc.sbuf_tensor([128, NMB * N_OUT], dt_in))
        pts = [
            ctx.enter_context(nc.psum_tensor(f"pt{m}", [MB, N_OUT], mybir.dt.float32))
            for m in range(NMB)
        ]
        ptw = ctx.enter_context(nc.psum_tensor("ptw", [MB, N_OUT], mybir.dt.float32))
        sem_gs = [ctx.enter_context(nc.semaphore(name=f"sem_g{g}"))
                  for g in range(len(PLAN))]
        sem_mm = ctx.enter_context(nc.semaphore(name="sem_mm"))
        sem_cp = ctx.enter_context(nc.semaphore(name="sem_cp"))
        sem_out = ctx.enter_context(nc.semaphore(name="sem_out"))

        engines = {"scalar": nc.scalar, "sync": nc.sync, "pool": nc.gpsimd}
        in_dmas = []
        for g, (eng, chunks) in enumerate(PLAN):
            k0, k1 = chunks[0], chunks[-1] + 1
            assert chunks == list(range(k0, k1))
            in_dmas.append(engines[eng].dma_start(
                out=insb[:, k0:k1, :], in_=in_t[:, k0:k1, :]
            ).then_inc(sem_gs[g], 16).ins)

        # PE pre-warm: dummy matmuls on (uninitialized) SBUF into a scratch
        # PSUM bank while waiting for the first gate, to ramp the PE p-state
        # before the real accumulation starts.
        for _ in range(26):
            nc.tensor.matmul(
                ptw[:], lhsT=insb[:, 0, 0:MB], rhs=insb[:, 0, NMB * MB:],
                start=True, stop=True, skip_group_check=True,
            )

        first_chunk = PLAN[0][1][0]
        last_chunk = PLAN[-1][1][-1]
        for g, (eng, chunks) in enumerate(PLAN):
            nc.tensor.wait_ge(sem_gs[g], 16)
            for k in chunks:
                for m in range(NMB):
                    mm = nc.tensor.matmul(
                        pts[m][:],
                        lhsT=insb[:, k, m * MB:(m + 1) * MB],
                        rhs=insb[:, k, NMB * MB:],
                        start=(k == first_chunk),
                        stop=(k == last_chunk),
                    )
                    if k == last_chunk:
                        if m == 0:
                            mm.then_inc(sem_mm, 1)
                        else:
                            nc.tensor.drain().then_inc(sem_mm, 1)

        # half-0 cast on DVE (overlaps the PE's half-1 tail), half-1 on the
        # ACT engine so the two casts run in parallel.
        nc.vector.wait_ge(sem_mm, 1)
        nc.vector.tensor_copy(osb[:, 0:N_OUT], pts[0][:])
        nc.vector.drain().then_inc(sem_cp, 1)
        nc.scalar.wait_ge(sem_mm, 2)
        nc.scalar.copy(osb[:, N_OUT:2 * N_OUT], pts[1][:])
        nc.scalar.drain().then_inc(sem_cp, 1)

        nc.sync.wait_ge(sem_cp, 2)
        nc.sync.dma_start(out=out_t[:], in_=osb[:]).then_inc(sem_out, 16)

        _hoist_first(nc, in_dmas)
    nc.compile()
    return nc


def _prep_inputs(x, W):
    xs = np.ascontiguousarray(x[..., 0], dtype=np.float32)
    W = np.asarray(W, dtype=np.float32)
    in_maps = []
    for c in range(NCORES):
        pr = slice(c * PL, (c + 1) * PL)
        xl = xs[:, pr, :].reshape(B, KL).T
        xl = xl.reshape(KCH, 128, B).transpose(1, 0, 2)
        wl = W[pr].transpose(0, 3, 1, 2).reshape(KL, N_OUT)
        wl = wl.reshape(KCH, 128, N_OUT).transpose(1, 0, 2)
        comb = np.concatenate([xl, wl], axis=2)
        in_maps.append({"in_t": np.ascontiguousarray(comb, dtype=np.float16)})
    return in_maps


def _squash(S):
    S = S.reshape(B, D, VD)
    sq = np.sum(S * S, axis=2, keepdims=True)
    v = S * sq / (1.0 + sq) / np.sqrt(sq + 1e-9)
    return v[..., None].astype(np.float32)


def run(x, W, trace=False):
    if "nc" not in _cache:
        _cache["nc"] = _build()
    nc = _cache["nc"]
    in_maps = _prep_inputs(x, W)
    try:
        res = run_bass_kernel_spmd(nc, in_maps, core_ids=list(range(NCORES)), trace=trace)
    except Exception:
        res = run_bass_kernel_spmd(nc, in_maps, core_ids=list(range(NCORES)), trace=trace)
    S = np.zeros((B, N_OUT), dtype=np.float32)
    for c in range(NCORES):
        o = res.results[c]["out"].astype(np.float32)
        S[:MB] += o[:, :N_OUT]
        S[MB:] += o[:, N_OUT:]
    return _squash(S), res


def kernel(x, W):
    out, _ = run(np.asarray(x), np.asarray(W))
    return out
